# revision 60
# baseline (speedup 1.0000x reference)
"""Trainium2 Bass kernel for nn_LIF_hh_neuron (B=2048, T=15, IN=512, C=1024).

Sharding: pure data-parallel over batch B across 8 NeuronCores (256 each).

Matmul precision scheme (1.5 PE-units instead of baseline's 3):
  p = x @ W computed as
    pass1: xh(f32r) @ Wh(f32r)        24 mm  (xh, Wh = round-to-11-bit-mantissa)
    pass2: e5m2(x - xh) @ e5m2(Wh)    12 DoubleRow mm at 0.5 cyc/row
  Both passes accumulate in ONE PSUM group (fp8 products land at natural
  scale, no combine op).  The W-rounding residual x@(W - Wh) is left
  uncorrected (~6e-5 std): measured end-to-end on the real inputs this gives
  1468 spike flips / 126M = rel err 0.0117 vs the 0.02 gate, deterministic.
  (A third e5m2 pass e5m2(x*2^-8)@e5m2((W-Wh)*2^8) recovers rel err 3.9e-3
  but the pipeline is DVE-bound, so it only costs startup time -- see
  kernel_3pass.py for that variant.)

Per-timestep LIF state update, engine-balanced (per (t,bt) tile; engine
ISA limits: Pool has no PSUM access and only TensorTensor; custom DVE ops
and PSUM reads must go to DVE; ACT does scale*x+bias activations only):
  DVE  : mem[h,:3] = ps_h + U_h           (TT from PSUM, 2x1536 el)
         U' = select(mem<=.8,.2*mem,0)+B  (custom op, 4096 el; B carries
                                           b1..b3 and bl so no extra bias add)
  Pool : mem[h,3] = z + U3                (TT, 2x512 el)
         z' = (v0+v1)+v2                  (TT, 2x1024 el)
  ACT  : S = Sign(mem - 0.8) -> int8      (4096 el; host maps {-1,0,1}->{0,1})
         v_w = wl_w * mem_w               (Identity w/ scale AP, 3x1024 el)
The U'/v/z' state updates for both bt tiles are emitted after both tiles'
mem/Sign/store so psum drains never queue behind the long custom op.
"""

import numpy as np
import ml_dtypes

import concourse.bass as bass
import concourse.mybir as mybir
import concourse.tile as tile
from concourse import bacc
from concourse import bass_utils
from concourse.dve_spec import Spec, Src0, Src1, C0, C1, Zero, select, lower
from concourse.dve_ops import has_src1, DveOp, OPS
import concourse.dve_ops as dve_ops_mod
from concourse.dve_uop import DveOpSpec

F32 = mybir.dt.float32
F32R = mybir.dt.float32r
F8E5 = mybir.dt.float8e5
U8 = mybir.dt.uint8
I8 = mybir.dt.int8
AluOp = mybir.AluOpType
DR = mybir.MatmulPerfMode.DoubleRow
E5 = ml_dtypes.float8_e5m2

B, T, IN, C = 2048, 15, 512, 1024
NCORES = 8
BLOC = B // NCORES          # 256 batches per core
NBT = BLOC // 128           # 2 batch tiles per core
KC = IN // 128              # 4 contraction chunks
NG = KC // 2                # 2 DoubleRow chunk-pairs
NH = 2                      # two c-halves of 512
THRESH = 0.8
DECAY = 0.2


def _register_op(name, spec, subdim=False):
    for existing in OPS:
        if existing.name == name:
            return existing
    op = DveOp(name, spec, subdim=subdim, uops_sha={})
    OPS.append(op)
    dve_ops_mod._SUB_OPCODE_FOR_NAME[name] = (
        dve_ops_mod._CUSTOM_DVE_ROW_BASE + len(OPS) - 1
    )
    dve_ops_mod.CUSTOM_DVE_SPECS[name] = spec
    shas = {}
    for ver in ("v3", "v4"):
        s = DveOpSpec(
            name=name,
            opcode=dve_ops_mod.get_dve_sub_opcode(name),
            uops=lower(spec, ver=ver),
            rd1_en=has_src1(spec),
        )
        shas[ver] = s.sha(ver)
    object.__setattr__(op, "uops_sha", shas)
    return op


# U' = select(mem <= thr, mem*decay, 0) + B
LIF_UPB = _register_op(
    "LIF_UPB_ANT",
    Spec(
        body=select(Src0 <= C0, Src0 * C1, Zero) + Src1,
        reference=lambda in0, in1, s0, s1, *_: (
            np.where(in0 <= s0, in0 * s1, 0.0) + in1
        ).astype(np.float32),
    ),
)


def _round11(a):
    """Round fp32 mantissa to 11 explicit bits (the f32r grid), nearest-even."""
    u = np.ascontiguousarray(a, np.float32).view(np.uint32)
    half = np.uint32(1 << 11)
    mask = np.uint32((1 << 12) - 1)
    frac = u & mask
    u2 = u & ~mask
    rup = (frac > half) | (
        (frac == half) & ((u2 >> np.uint32(12)) & np.uint32(1)).astype(bool)
    )
    return (u2 + np.where(rup, np.uint32(1 << 12), np.uint32(0))).view(np.float32)


def _build(nt=T):
    nc = bacc.Bacc("TRN2", target_bir_lowering=False, debug=False)

    # x operands: [T, NBT, ...] per-core pre-transposed on host
    d_xhT = nc.dram_tensor("xhT", [nt, NBT, KC, 128, 128], F32, kind="ExternalInput").ap()
    d_xl8 = nc.dram_tensor("xl8", [nt, NBT, 128, NG, 2, 128], U8, kind="ExternalInput").ap()
    d_wh = nc.dram_tensor("wh", [KC, 128, 3, C], F32, kind="ExternalInput").ap()
    d_wh8 = nc.dram_tensor("wh8", [128, NG, 2, 3, C], U8, kind="ExternalInput").ap()
    d_bB = nc.dram_tensor("bB", [NH, 4, 512], F32, kind="ExternalInput").ap()
    d_wlb = nc.dram_tensor("wlb", [1, 5], F32, kind="ExternalInput").ap()
    d_out = nc.dram_tensor("spk", [BLOC, nt, 4 * C], I8, kind="ExternalOutput").ap()

    with tile.TileContext(nc) as tc:
        with (
            tc.tile_pool(name="wpool", bufs=1) as wpool,
            tc.tile_pool(name="state", bufs=1) as state,
            tc.tile_pool(name="mem", bufs=2) as mempool,
            tc.tile_pool(name="ztmp", bufs=1) as zpool,
            tc.tile_pool(name="spool", bufs=2) as spool,
            tc.tile_pool(name="xin", bufs=2) as xin,
            tc.tile_pool(name="pspool", bufs=2, space="PSUM") as pspool,
        ):
            # ---- static tiles ----
            # weight preloads spread across engine DMA queues so they run in
            # parallel with each other and with the per-tile x loads (sync)
            # wh chunk 0 first on the sync queue (the scalar queue pays a
            # ~1.3us act-table load before it can issue); k1/k2 on scalar,
            # k3 on sync after the first x loads -- PE consumes k-ordered
            t_wh = wpool.tile([128, KC, 3, C], F32R, tag="wh")
            nc.sync.dma_start(out=t_wh[:, 0], in_=d_wh[0].bitcast(F32R))
            for k in range(1, 3):
                nc.scalar.dma_start(out=t_wh[:, k], in_=d_wh[k].bitcast(F32R))
            t_wh8 = wpool.tile([128, NG, 2, 3, C], F8E5, tag="wh8")
            nc.gpsimd.dma_start(out=t_wh8, in_=d_wh8.bitcast(F8E5))
            t_B = wpool.tile([128, NH, 4, 512], F32, tag="B")
            nc.gpsimd.dma_start(
                out=t_B,
                in_=bass.AP(tensor=d_bB.tensor, offset=0, ap=[[0, 128], [1, NH * 4 * 512]]),
            )
            t_wlb = wpool.tile([128, 5], F32, tag="wlb")
            nc.gpsimd.dma_start(
                out=t_wlb,
                in_=bass.AP(tensor=d_wlb.tensor, offset=0, ap=[[0, 128], [1, 5]]),
            )

            # ---- per-bt recurrent state ----
            t_U = [
                state.tile([128, NH, 4, 512], F32, tag=f"U{bt}", name=f"U{bt}")
                for bt in range(NBT)
            ]
            t_z = [
                state.tile([128, NH, 512], F32, tag=f"z{bt}", name=f"z{bt}")
                for bt in range(NBT)
            ]
            for bt in range(NBT):
                # t=0 reads biases straight from t_B; only z needs zeroing
                nc.vector.memset(t_z[bt], 0.0)

            for t in range(nt):
                mems = [None] * NBT
                for bt in range(NBT):
                    b0 = bt * 128
                    xh = xin.tile([128, KC, 128], F32R, tag="xh")
                    nc.sync.dma_start(
                        out=xh,
                        in_=d_xhT[t, bt].rearrange("k p b -> p k b").bitcast(F32R),
                    )
                    xl8 = xin.tile([128, NG, 2, 128], F8E5, tag="xl8")
                    nc.sync.dma_start(out=xl8, in_=d_xl8[t, bt].bitcast(F8E5))
                    if t == 0 and bt == 0:
                        nc.sync.dma_start(out=t_wh[:, 3], in_=d_wh[3].bitcast(F32R))

                    mem = mempool.tile(
                        [128, NH, 4, 512], F32, tag="mem", name=f"mem_{t}_{bt}"
                    )
                    mems[bt] = mem
                    S8 = spool.tile([128, NH, 512, 4], I8, tag="S")

                    for h in range(NH):
                        c0 = h * 512
                        ps = pspool.tile(
                            [128, 3, 512], F32, tag="ps", name=f"ps_{t}_{bt}_{h}"
                        )
                        for k in range(KC):
                            for w in range(3):
                                nc.tensor.matmul(
                                    ps[:, w, :],
                                    xh[:, k, :],
                                    t_wh[:, k, w, c0 : c0 + 512],
                                    start=(k == 0),
                                    stop=False,
                                )
                        for lhs, wt, last in ((xl8, t_wh8, True),):
                            for g in range(NG):
                                for w in range(3):
                                    nc.tensor.matmul(
                                        ps[:, w, :],
                                        lhs[:, g],
                                        wt[:, g, :, w, c0 : c0 + 512],
                                        start=False,
                                        stop=(last and g == NG - 1),
                                        perf_mode=DR,
                                    )
                        # Pool: plane 3 = z + U3 (all SBUF; Pool TT only)
                        ub = t_B if t == 0 else t_U[bt]
                        nc.gpsimd.tensor_tensor(
                            out=mem[:, h, 3, :],
                            in0=t_z[bt][:, h],
                            in1=ub[:, h, 3, :],
                            op=AluOp.add,
                        )
                        # memadd (DVE; Pool cannot read PSUM) -> Sign (ACT)
                        # -> store, split in c-halves on the last tile so the
                        # tail drains pipelined
                        nsp = 4 if (t == nt - 1 and h == NH - 1) else 1
                        cw = 512 // nsp
                        for s in range(nsp):
                            ca, cb = s * cw, (s + 1) * cw
                            nc.vector.tensor_tensor(
                                out=mem[:, h, 0:3, ca:cb],
                                in0=ps[:, :, ca:cb],
                                in1=ub[:, h, 0:3, ca:cb],
                                op=AluOp.add,
                            )
                            mem_rd = bass.AP(
                                tensor=mem.tensor,
                                offset=mem.offset + h * 4 * 512 + ca,
                                ap=[mem.ap[0], [1, cw], [512, 4]],
                            )
                            nc.scalar.activation(
                                S8[:, h, ca:cb],
                                mem_rd,
                                mybir.ActivationFunctionType.Sign,
                                bias=t_wlb[:, 4:5],
                                scale=1.0,
                            )
                            nc.sync.dma_start(
                                out=d_out[
                                    b0 : b0 + 128,
                                    t,
                                    h * 2048 + ca * 4 : h * 2048 + cb * 4,
                                ],
                                in_=S8[:, h, ca:cb].rearrange("p c w -> p (c w)"),
                            )

                # state updates emitted AFTER both bt tiles so the DVE's
                # psum-drain memadds never queue behind the long Unext op
                if t == nt - 1:
                    continue
                for bt in range(NBT):
                    mem = mems[bt]
                    # DVE custom: U' = select(mem<=thr, decay*mem, 0) + B
                    nc.vector._custom_dve(
                        LIF_UPB,
                        out=t_U[bt].rearrange("p h w c -> p (h w c)"),
                        in0=mem.rearrange("p h w c -> p (h w c)"),
                        in1=t_B.rearrange("p h w c -> p (h w c)"),
                        s0=THRESH,
                        s1=DECAY,
                    )
                    # z' = wl0*mem0 + wl1*mem1 + wl2*mem2:
                    # scaled copies on ACT, adds on Pool (real-HW GPSIMD
                    # Add/Multiply run at 0.42 efficiency -- keep Pool light)
                    v = [None] * 3
                    for w in range(3):
                        v[w] = zpool.tile(
                            [128, NH, 512], F32, tag=f"v{w}", name=f"v{w}_{t}_{bt}"
                        )
                        nc.scalar.activation(
                            v[w],
                            mem[:, :, w, :],
                            mybir.ActivationFunctionType.Identity,
                            bias=0.0,
                            scale=t_wlb[:, w : w + 1],
                        )
                    zt = zpool.tile([128, NH, 512], F32, tag="zt")
                    nc.gpsimd.tensor_tensor(
                        out=zt, in0=v[0], in1=v[1], op=AluOp.add
                    )
                    nc.gpsimd.tensor_tensor(
                        out=t_z[bt], in0=zt, in1=v[2], op=AluOp.add
                    )

    nc.finalize()
    return nc


_NC_CACHE = {}


def _get_nc(nt=T):
    if nt not in _NC_CACHE:
        _NC_CACHE[nt] = _build(nt)
    return _NC_CACHE[nt]


def prepare_inputs(inputs):
    """Host-side preprocessing: returns per-core in_maps for the bass kernel."""
    x = np.ascontiguousarray(np.asarray(inputs["x"], dtype=np.float32))
    W = [np.asarray(inputs[f"W{i}"], dtype=np.float32) for i in (1, 2, 3)]
    bvec = [np.asarray(inputs[f"b{i}"], dtype=np.float32) for i in (1, 2, 3)]
    Wl = np.asarray(inputs["Wl"], dtype=np.float32)
    bl = np.asarray(inputs["bl"], dtype=np.float32)

    WT = np.stack([Wk.T for Wk in W], axis=1).astype(np.float32)  # [IN, 3, C]
    Wh = _round11(WT)
    Wlr = WT - Wh
    wh = np.ascontiguousarray(Wh.reshape(KC, 128, 3, C))
    # fp8 weight operands, [128(p), NG, 2(s), 3, C]
    def wlayout(a):
        return np.ascontiguousarray(
            a.reshape(NG, 2, 128, 3, C).transpose(2, 0, 1, 3, 4)
        )

    wh8 = wlayout(Wh.astype(E5).view(np.uint8))

    # bias tile [NH, 4, 512]: planes 0-2 hold b_w, plane 3 holds bl
    bB = np.zeros((NH, 4, 512), np.float32)
    for w in range(3):
        bB[:, w, :] = bvec[w].reshape(NH, 512)
    bB[:, 3, :] = bl[0]
    wlb = np.concatenate(
        [Wl[0].reshape(3), bl.reshape(1), np.float32([-THRESH])]
    ).reshape(1, 5).astype(np.float32)

    xh = _round11(x)
    xl = x - xh

    # f32r operand: [NC, T, NBT, KC, 128(k), 128(b)]
    a = xh.reshape(NCORES, NBT, 128, T, KC, 128)
    xhT = np.ascontiguousarray(a.transpose(0, 3, 1, 4, 5, 2))
    # fp8 operands: [NC, T, NBT, 128(p), NG, 2(s), 128(b)]
    def x8layout(a8):
        a8 = a8.reshape(NCORES, NBT, 128, T, NG, 2, 128)
        return np.ascontiguousarray(a8.transpose(0, 3, 1, 6, 4, 5, 2))

    xl8 = x8layout(xl.astype(E5).view(np.uint8))

    in_maps = [
        dict(
            xhT=xhT[c],
            xl8=xl8[c],
            wh=wh,
            wh8=wh8,
            bB=bB,
            wlb=wlb,
        )
        for c in range(NCORES)
    ]
    return in_maps


def kernel(**inputs):
    nc = _get_nc()
    in_maps = prepare_inputs(inputs)
    res = bass_utils.run_bass_kernel_spmd(nc, in_maps, core_ids=list(range(NCORES)))
    # Sign output: +1 -> spike, {0, -1 (or wrapped 255)} -> no spike
    out = np.concatenate(
        [(r["spk"] == 1).astype(np.float32) for r in res.results], axis=0
    )
    return out


if __name__ == "__main__":
    rng = np.random.default_rng(0)
    s_in = 1.0 / np.sqrt(IN)
    s3 = 1.0 / np.sqrt(3.0)
    ins = dict(
        x=rng.standard_normal((B, T, IN)).astype(np.float32),
        W1=rng.uniform(-s_in, s_in, (C, IN)).astype(np.float32),
        b1=rng.uniform(-s_in, s_in, (C,)).astype(np.float32),
        W2=rng.uniform(-s_in, s_in, (C, IN)).astype(np.float32),
        b2=rng.uniform(-s_in, s_in, (C,)).astype(np.float32),
        W3=rng.uniform(-s_in, s_in, (C, IN)).astype(np.float32),
        b3=rng.uniform(-s_in, s_in, (C,)).astype(np.float32),
        Wl=rng.uniform(-s3, s3, (1, 3)).astype(np.float32),
        bl=rng.uniform(-s3, s3, (1,)).astype(np.float32),
        wins=T,
    )
    out = kernel(**ins)

    # numpy reference
    p = [
        (ins["x"].reshape(B * T, IN) @ ins[f"W{k+1}"].T + ins[f"b{k+1}"]).reshape(
            B, T, C
        )
        for k in range(3)
    ]
    mem = np.zeros((B, C, 4), np.float32)
    spk = np.zeros((B, C, 4), np.float32)
    exp = np.zeros((B, T, C, 4), np.float32)
    for t in range(T):
        inner = mem[..., :3] @ ins["Wl"][0] + ins["bl"][0]
        ia = np.stack([p[0][:, t], p[1][:, t], p[2][:, t], inner], axis=-1)
        mem = mem * np.float32(0.2) * (1.0 - spk) + ia
        spk = (mem > 0.8).astype(np.float32)
        exp[:, t] = spk
    exp = exp.reshape(B, T, C * 4)
    rel = np.linalg.norm(out - exp) / np.linalg.norm(exp)
    print("out", out.shape, out.dtype, "density", out.mean())
    print("rel err vs numpy fp32:", rel, "nflips", np.abs(out - exp).sum())
